# revision 11
# baseline (speedup 1.0000x reference)
"""Trainium2 Bass kernel for nn_DGNN (gnn_message_passing).

Reference computation (B=4, N=8192, F=32):
    delay_steps = time_delay // 5
    active      = (t >= delay_steps) & (adj > 0)
    A           = where(active, adj, 0)              # == adj * (time_delay <= 5*t+4)
    adjusted    = einsum('ij,bjf->bif', A, x)
    h           = relu(adjusted @ W1 + b1)
    out         = sigmoid(h @ W2 + b2)

Sharding / layout (host does layout-only transforms + dtype container
changes, no reference math):
  - destination nodes i are split row-wise across 8 cores (1024 each);
  - adj/time_delay are shipped transposed ([j, i], j on partitions) because
    the PE contracts over the partition dim;
  - adj and x are shipped as fp16 (the matmul runs in fp16 with fp32 PSUM
    accumulation; measured end-to-end rel err ~4e-3 vs the 2e-2 gate, and
    it halves both HBM traffic and PE time vs fp32);
  - time_delay values are 0..99 so they are shipped as int8 when they fit
    (lossless narrowing; falls back to int32 otherwise);
  - adjT/tdT are laid out chunk-linear ([P, jt*ni] with whole j-tiles
    contiguous) so each DMA moves multiple j-tiles in one 1-2 MiB
    transfer (large transfers amortize the per-DMA completion latency);
  - x is repacked so the 4 batches sit side-by-side in the stationary
    operand (partition q = 32*b + f), giving full-width M=128 matmuls;
  - W1/W2 become 128x128 block-diagonal so the per-node MLP handles all 4
    batches in one matmul.

On-device per core: stream adjT/tdT chunks double-buffered on the two
HWDGE rings, one fused DVE op per chunk (TENSOR_MASK:
out = select(td < thr+0.5, adj, 0)) produces the masked adjacency in
fp16, fp16 matmuls accumulate adjusted^T over 64 K-tiles in fp32 PSUM,
then the block-diagonal MLP (fp16 matmuls, fp32 bias/activations) and
sigmoid run on-chip. Output returns transposed per core and is unsharded
on the host. The first two chunks are half-size so compute starts ~5 us
earlier in the DMA stream.
"""

import numpy as np

B = 4
N = 8192
F = 32
P = 128
NCORES = 8
NI = N // NCORES  # dest-nodes per core
JT = N // P       # contraction tiles

MM_N = 512        # moving-operand free dim per matmul (one PSUM bank)
CHUNK_JT = 4      # j-tiles per DMA chunk in steady state
IO_BUFS = 4       # in-flight adj/td chunk buffers
A_BUFS = 3        # in-flight masked-adjacency buffers


def _chunk_schedule(jt_n):
    """(start_tile, n_tiles) per DMA chunk: two half chunks to ramp the
    pipeline, then full chunks."""
    half = CHUNK_JT // 2
    sched = []
    pos = 0
    if jt_n >= 2 * CHUNK_JT:
        sched = [(0, half), (half, half)]
        pos = 2 * half
    while pos < jt_n:
        ln = min(CHUNK_JT, jt_n - pos)
        sched.append((pos, ln))
        pos += ln
    return sched


def _round_fp32r(a):
    """Round fp32 to the fp32r grid (11 explicit mantissa bits, RNE)."""
    u = np.ascontiguousarray(a, dtype=np.float32).view(np.uint32)
    low = u & np.uint32(0xFFF)
    lsb = (u >> np.uint32(12)) & np.uint32(1)
    roundup = (low > 0x800) | ((low == 0x800) & (lsb == 1))
    u2 = (u & np.uint32(0xFFFFF000)) + (roundup.astype(np.uint32) << np.uint32(12))
    return u2.view(np.float32)


def _build(nj, ni, thr, mm_dtype_name="float16", td_dtype=np.int8):
    """Trace + compile the per-core Bass program."""
    from contextlib import ExitStack

    import concourse.bacc as bacc
    import concourse.mybir as mybir
    import concourse.tile as tile

    f32 = mybir.dt.float32
    mm_dt = getattr(mybir.dt, mm_dtype_name)
    td_dt = mybir.dt.from_np(np.dtype(td_dtype))

    jt_n = nj // P
    mm_n = min(MM_N, ni)
    nh = ni // mm_n
    sched = _chunk_schedule(jt_n)

    nc = bacc.Bacc("TRN2", target_bir_lowering=False, debug=False)

    adjT_d = nc.dram_tensor("adjT", [P, jt_n * ni], mm_dt, kind="ExternalInput").ap()
    tdT_d = nc.dram_tensor("tdT", [P, jt_n * ni], td_dt, kind="ExternalInput").ap()
    xsb_d = nc.dram_tensor("xsb", [P, jt_n * P], mm_dt, kind="ExternalInput").ap()
    bd1_d = nc.dram_tensor("bd1", [P, P], mm_dt, kind="ExternalInput").ap()
    bd2_d = nc.dram_tensor("bd2", [P, P], mm_dt, kind="ExternalInput").ap()
    bias1_d = nc.dram_tensor("bias1", [P, 1], f32, kind="ExternalInput").ap()
    bias2_d = nc.dram_tensor("bias2", [P, 1], f32, kind="ExternalInput").ap()
    outT_d = nc.dram_tensor("outT", [P, ni], f32, kind="ExternalOutput").ap()

    with tile.TileContext(nc) as tc, ExitStack() as ctx:
        io = ctx.enter_context(tc.tile_pool(name="io", bufs=IO_BUFS))
        wrk = ctx.enter_context(tc.tile_pool(name="wrk", bufs=A_BUFS))
        singles = ctx.enter_context(tc.tile_pool(name="singles", bufs=1))
        pp = ctx.enter_context(tc.tile_pool(name="pp", bufs=1, space="PSUM"))

        x_t = singles.tile([P, jt_n * P], mm_dt)
        psum_main = pp.tile([P, ni], f32)
        bd1_t = singles.tile([P, P], mm_dt)
        bd2_t = singles.tile([P, P], mm_dt)
        bias1_t = singles.tile([P, 1], f32)
        bias2_t = singles.tile([P, 1], f32)
        warm_t = singles.tile([P, 1], f32)

        # ACT table pre-warm for the sigmoid (used both as the exact 0/1
        # step builder in the stream and in the tail), before any data
        # lands (no input dependencies).
        bias_thr_t = singles.tile([P, 1], f32)
        nc.vector.memset(bias_thr_t, 100.0 * (float(thr) + 0.5))
        nc.vector.memset(warm_t, 0.0)
        nc.scalar.activation(
            warm_t, warm_t, mybir.ActivationFunctionType.Sigmoid, bias=bias_thr_t
        )

        def tt_mult(eng, out, in0, in1):
            # plain TensorTensor multiply (not exposed by the engine API);
            # with all-2B operands it is 2x_1P-eligible on the DVE
            return eng.add_instruction(
                mybir.InstTensorTensor(
                    name=nc.get_next_instruction_name(),
                    op=mybir.AluOpType.mult,
                    ins=[eng.lower_ap(in0), eng.lower_ap(in1)],
                    outs=[eng.lower_ap(out)],
                )
            )

        for c, (j0, ln) in enumerate(sched):
            # all stream DMAs ride the SP HWDGE ring; the ACT ring only
            # carries small constants so the scalar engine stays free for
            # the mask compares
            cs = slice(j0 * ni, (j0 + ln) * ni)
            adj_t = io.tile([P, CHUNK_JT * ni], mm_dt, tag="adj")
            nc.sync.dma_start(out=adj_t[:, : ln * ni], in_=adjT_d[:, cs])
            td_t = io.tile([P, CHUNK_JT * ni], td_dt, tag="td")
            nc.sync.dma_start(out=td_t[:, : ln * ni], in_=tdT_d[:, cs])
            xs = slice(j0 * P, (j0 + ln) * P)
            nc.sync.dma_start(out=x_t[:, xs], in_=xsb_d[:, xs])
            if c == 1:
                # small constants, off the critical path
                nc.scalar.dma_start(out=bd1_t, in_=bd1_d)
                nc.scalar.dma_start(out=bd2_t, in_=bd2_d)
                nc.scalar.dma_start(out=bias1_t, in_=bias1_d)
                nc.scalar.dma_start(out=bias2_t, in_=bias2_d)

            # mask, load-balanced across two engines: most chunks build the
            # exact 0/1 step on the ACT engine (sigmoid saturates hard:
            # |z| >= 50 gives exactly 1.0 / 0.0) and multiply on the DVE
            # with an all-fp16 TensorTensor; every 4th chunk runs entirely
            # on the DVE as one fused (td <= thr) * adj op so neither
            # engine becomes the serial bottleneck.
            a_t = wrk.tile([P, CHUNK_JT * ni], mm_dt, tag="a")
            if c % 4 == 3:
                nc.vector.scalar_tensor_tensor(
                    out=a_t[:, : ln * ni],
                    in0=td_t[:, : ln * ni],
                    scalar=float(thr) + 0.5,
                    in1=adj_t[:, : ln * ni],
                    op0=mybir.AluOpType.is_lt,
                    op1=mybir.AluOpType.mult,
                )
            else:
                s_t = wrk.tile([P, CHUNK_JT * ni], mm_dt, tag="s")
                nc.scalar.activation(
                    s_t[:, : ln * ni],
                    td_t[:, : ln * ni],
                    mybir.ActivationFunctionType.Sigmoid,
                    bias=bias_thr_t,
                    scale=-100.0,
                )
                tt_mult(
                    nc.vector,
                    a_t[:, : ln * ni],
                    s_t[:, : ln * ni],
                    adj_t[:, : ln * ni],
                )

            for jl in range(ln):
                jt = j0 + jl
                lhsT = x_t[:, jt * P : (jt + 1) * P]
                for h in range(nh):
                    nc.tensor.matmul(
                        psum_main[:, h * mm_n : (h + 1) * mm_n],
                        lhsT,
                        a_t[:, jl * ni + h * mm_n : jl * ni + (h + 1) * mm_n],
                        start=(jt == 0),
                        stop=(jt == jt_n - 1),
                    )

        # Per-node MLP, pipelined in independent column halves.
        h_ps = pp.tile([P, ni], f32, tag="hps")
        o_ps = pp.tile([P, ni], f32, tag="ops")
        for h in range(nh):
            hs = slice(h * mm_n, (h + 1) * mm_n)
            res_t = singles.tile([P, mm_n], mm_dt, tag=f"res{h}", name=f"res{h}")
            nc.vector.tensor_copy(res_t, psum_main[:, hs])
            nc.tensor.matmul(h_ps[:, hs], bd1_t, res_t, start=True, stop=True)
            # h = relu(. + b1) fused on DVE: (in + bias) max 0
            h_t = singles.tile([P, mm_n], mm_dt, tag=f"h{h}", name=f"h{h}")
            nc.vector.tensor_scalar(
                h_t, h_ps[:, hs], bias1_t, 0.0,
                op0=mybir.AluOpType.add,
                op1=mybir.AluOpType.max,
            )
            nc.tensor.matmul(o_ps[:, hs], bd2_t, h_t, start=True, stop=True)
            out_t = singles.tile([P, mm_n], f32, tag=f"out{h}", name=f"out{h}")
            nc.scalar.activation(
                out_t, o_ps[:, hs], mybir.ActivationFunctionType.Sigmoid, bias=bias2_t
            )
            nc.sync.dma_start(out=outT_d[:, hs], in_=out_t)

    nc.compile()
    return nc


def _host_prep(x, adj, time_delay, t, W1, b1, W2, b2, ncores, mm_np, rnd, td_dtype):
    """Layout-only transforms (transpose / repack / dtype container changes)."""
    x = np.ascontiguousarray(np.asarray(x, dtype=np.float32))
    adj = np.asarray(adj, dtype=np.float32)
    td = np.asarray(time_delay)
    b, n, f = x.shape
    ni = n // ncores
    jt_n = n // P

    thr = int(t) * 5 + 4  # time_delay // 5 <= t  <=>  time_delay <= 5t+4

    # chunk-linear transposed layouts: arr[p, jt, i_global]
    adjT = np.ascontiguousarray(
        rnd(adj.T).reshape(jt_n, P, n).transpose(1, 0, 2)
    )
    tdT = np.ascontiguousarray(
        td.T.astype(td_dtype).reshape(jt_n, P, n).transpose(1, 0, 2)
    )
    # stationary x: x_sb[p, jt*P + 32*b + f] = x[b, jt*P + p, f]
    xsb = rnd(
        x.reshape(b, jt_n, P, f).transpose(2, 1, 0, 3).reshape(P, jt_n * b * f)
    )
    bd1 = np.zeros((P, P), np.float32)
    bd2 = np.zeros((P, P), np.float32)
    for bb in range(b):
        bd1[bb * f : (bb + 1) * f, bb * f : (bb + 1) * f] = W1
        bd2[bb * f : (bb + 1) * f, bb * f : (bb + 1) * f] = W2
    bd1 = rnd(bd1)
    bd2 = rnd(bd2)
    bias1 = np.ascontiguousarray(np.tile(np.asarray(b1, np.float32), b).reshape(P, 1))
    bias2 = np.ascontiguousarray(np.tile(np.asarray(b2, np.float32), b).reshape(P, 1))

    in_maps = []
    for c in range(ncores):
        sl = slice(c * ni, (c + 1) * ni)
        in_maps.append(
            {
                "adjT": np.ascontiguousarray(adjT[:, :, sl]).reshape(P, jt_n * ni),
                "tdT": np.ascontiguousarray(tdT[:, :, sl]).reshape(P, jt_n * ni),
                "xsb": xsb,
                "bd1": bd1,
                "bd2": bd2,
                "bias1": bias1,
                "bias2": bias2,
            }
        )
    return thr, in_maps


def _run(x, adj, time_delay, t, W1, b1, W2, b2, ncores=NCORES,
         mm_dtype_name="float16", trace=False):
    from concourse.bass_utils import run_bass_kernel_spmd

    b, n, f = np.asarray(x).shape
    ni = n // ncores
    td = np.asarray(time_delay)
    # int8 shipping is only a container change; keep int32 when values
    # (or the threshold compare range) would not fit exactly.
    thr_chk = int(t) * 5 + 4
    if td.min() >= -127 and td.max() <= 127 and -127 <= thr_chk <= 127:
        td_dtype = np.int8
    else:
        td_dtype = np.int32
    if mm_dtype_name == "float32r":
        rnd = _round_fp32r
        mm_np = np.float32
    elif mm_dtype_name == "float16":
        mm_np = np.float16
        rnd = lambda a: np.ascontiguousarray(a, dtype=np.float16)
    else:
        mm_np = np.float32
        rnd = lambda a: np.ascontiguousarray(a, dtype=np.float32)
    thr, in_maps = _host_prep(
        x, adj, time_delay, t, W1, b1, W2, b2, ncores, mm_np, rnd, td_dtype
    )
    nc = _build(n, ni, thr, mm_dtype_name, td_dtype)
    res = run_bass_kernel_spmd(
        nc, in_maps, core_ids=list(range(ncores)), trace=trace
    )
    full = np.concatenate([r["outT"] for r in res.results], axis=1)  # [P, n]
    out = np.ascontiguousarray(full.reshape(b, f, n).transpose(0, 2, 1))
    return out, res


def kernel(x, adj, time_delay, t, W1, b1, W2, b2):
    out, _ = _run(x, adj, time_delay, t, W1, b1, W2, b2)
    return out


# revision 15
# speedup vs baseline: 1.0809x; 1.0809x over previous
"""Trainium2 Bass kernel for nn_DGNN (gnn_message_passing).

Reference computation (B=4, N=8192, F=32):
    delay_steps = time_delay // 5
    active      = (t >= delay_steps) & (adj > 0)
    A           = where(active, adj, 0)              # == adj * (time_delay <= 5*t+4)
    adjusted    = einsum('ij,bjf->bif', A, x)
    h           = relu(adjusted @ W1 + b1)
    out         = sigmoid(h @ W2 + b2)

Sharding / layout (host does layout-only transforms + dtype container
changes, no reference math):
  - destination nodes i are split row-wise across 8 cores (1024 each);
  - adj/time_delay are shipped transposed ([j, i], j on partitions) because
    the PE contracts over the partition dim;
  - adj and x are shipped as fp16 (the matmul runs in fp16 with fp32 PSUM
    accumulation; measured end-to-end rel err ~4e-3 vs the 2e-2 gate, and
    it halves both HBM traffic and PE time vs fp32);
  - time_delay values are 0..99 so they are shipped as int8 when they fit
    (lossless narrowing; falls back to int32 otherwise);
  - adjT/tdT are laid out chunk-linear ([P, jt*ni] with whole j-tiles
    contiguous) so each DMA moves multiple j-tiles in one 1-2 MiB
    transfer (large transfers amortize the per-DMA completion latency);
  - x is repacked so the 4 batches sit side-by-side in the stationary
    operand (partition q = 32*b + f), giving full-width M=128 matmuls;
  - W1/W2 become 128x128 block-diagonal so the per-node MLP handles all 4
    batches in one matmul.

On-device per core: stream adjT/tdT chunks double-buffered on the two
HWDGE rings, one fused DVE op per chunk (TENSOR_MASK:
out = select(td < thr+0.5, adj, 0)) produces the masked adjacency in
fp16, fp16 matmuls accumulate adjusted^T over 64 K-tiles in fp32 PSUM,
then the block-diagonal MLP (fp16 matmuls, fp32 bias/activations) and
sigmoid run on-chip. Output returns transposed per core and is unsharded
on the host. The first two chunks are half-size so compute starts ~5 us
earlier in the DMA stream.
"""

import numpy as np

B = 4
N = 8192
F = 32
P = 128
NCORES = 8
NI = N // NCORES  # dest-nodes per core
JT = N // P       # contraction tiles

MM_N = 512        # moving-operand free dim per matmul (one PSUM bank)
CHUNK_JT = 4      # j-tiles per DMA chunk in steady state
IO_BUFS = 6       # in-flight adj/td chunk buffers
A_BUFS = 3        # in-flight masked-adjacency buffers


def _chunk_schedule(jt_n):
    """(start_tile, n_tiles) per DMA chunk: half chunks at both ends (fast
    pipeline ramp, fast drain), full chunks in the middle."""
    half = CHUNK_JT // 2
    sched = []
    pos = 0
    if jt_n >= 4 * CHUNK_JT:
        sched = [(0, half), (half, half)]
        pos = 2 * half
        tail_small = 2
    else:
        tail_small = 0
    end = jt_n - tail_small * half
    while pos < end:
        ln = min(CHUNK_JT, end - pos)
        sched.append((pos, ln))
        pos += ln
    for _ in range(tail_small):
        sched.append((pos, half))
        pos += half
    return sched


def _round_fp32r(a):
    """Round fp32 to the fp32r grid (11 explicit mantissa bits, RNE)."""
    u = np.ascontiguousarray(a, dtype=np.float32).view(np.uint32)
    low = u & np.uint32(0xFFF)
    lsb = (u >> np.uint32(12)) & np.uint32(1)
    roundup = (low > 0x800) | ((low == 0x800) & (lsb == 1))
    u2 = (u & np.uint32(0xFFFFF000)) + (roundup.astype(np.uint32) << np.uint32(12))
    return u2.view(np.float32)


def _build(nj, ni, thr, mm_dtype_name="float16", td_dtype=np.int8):
    """Trace + compile the per-core Bass program."""
    from contextlib import ExitStack

    import concourse.bacc as bacc
    import concourse.mybir as mybir
    import concourse.tile as tile

    f32 = mybir.dt.float32
    mm_dt = getattr(mybir.dt, mm_dtype_name)
    td_dt = mybir.dt.from_np(np.dtype(td_dtype))

    jt_n = nj // P
    mm_n = min(MM_N, ni)
    nh = ni // mm_n
    sched = _chunk_schedule(jt_n)

    nc = bacc.Bacc("TRN2", target_bir_lowering=False, debug=False)

    adjT_d = nc.dram_tensor("adjT", [P, jt_n * ni], mm_dt, kind="ExternalInput").ap()
    tdT_d = nc.dram_tensor("tdT", [P, jt_n * ni], td_dt, kind="ExternalInput").ap()
    xsb_d = nc.dram_tensor("xsb", [P, jt_n * P], mm_dt, kind="ExternalInput").ap()
    bd1_d = nc.dram_tensor("bd1", [P, P], mm_dt, kind="ExternalInput").ap()
    bd2_d = nc.dram_tensor("bd2", [P, P], mm_dt, kind="ExternalInput").ap()
    bias1_d = nc.dram_tensor("bias1", [P, 1], f32, kind="ExternalInput").ap()
    bias2_d = nc.dram_tensor("bias2", [P, 1], f32, kind="ExternalInput").ap()
    outT_d = nc.dram_tensor("outT", [P, ni], f32, kind="ExternalOutput").ap()

    with tile.TileContext(nc) as tc, ExitStack() as ctx:
        io = ctx.enter_context(tc.tile_pool(name="io", bufs=IO_BUFS))
        wrk = ctx.enter_context(tc.tile_pool(name="wrk", bufs=A_BUFS))
        singles = ctx.enter_context(tc.tile_pool(name="singles", bufs=1))
        pp = ctx.enter_context(tc.tile_pool(name="pp", bufs=1, space="PSUM"))

        x_t = singles.tile([P, jt_n * P], mm_dt)
        psum_main = pp.tile([P, ni], f32)
        bd1_t = singles.tile([P, P], mm_dt)
        bd2_t = singles.tile([P, P], mm_dt)
        bias1_t = singles.tile([P, 1], f32)
        bias2_t = singles.tile([P, 1], f32)
        warm_t = singles.tile([P, 1], f32)

        # ACT table pre-warm for the sigmoid (used both as the exact 0/1
        # step builder in the stream and in the tail), before any data
        # lands (no input dependencies).
        bias_thr_t = singles.tile([P, 1], f32)
        nc.vector.memset(bias_thr_t, 100.0 * (float(thr) + 0.5))
        nc.vector.memset(warm_t, 0.0)
        nc.scalar.activation(
            warm_t, warm_t, mybir.ActivationFunctionType.Sigmoid, bias=bias_thr_t
        )

        for c, (j0, ln) in enumerate(sched):
            # all stream DMAs ride the SP HWDGE ring; the ACT ring only
            # carries small constants so the scalar engine stays free for
            # the mask compares
            cs = slice(j0 * ni, (j0 + ln) * ni)
            adj_t = io.tile([P, CHUNK_JT * ni], mm_dt, tag="adj")
            nc.sync.dma_start(out=adj_t[:, : ln * ni], in_=adjT_d[:, cs])
            td_t = io.tile([P, CHUNK_JT * ni], td_dt, tag="td")
            nc.sync.dma_start(out=td_t[:, : ln * ni], in_=tdT_d[:, cs])
            xs = slice(j0 * P, (j0 + ln) * P)
            nc.sync.dma_start(out=x_t[:, xs], in_=xsb_d[:, xs])
            if c == 1:
                # small constants, off the critical path
                nc.scalar.dma_start(out=bd1_t, in_=bd1_d)
                nc.scalar.dma_start(out=bd2_t, in_=bd2_d)
                nc.scalar.dma_start(out=bias1_t, in_=bias1_d)
                nc.scalar.dma_start(out=bias2_t, in_=bias2_d)

            # A = (td < thr + 0.5) * adj in one fused DVE op. Keeping the
            # mask as a single 1x op minimizes total engine activity: the
            # chip's activity throttle (HAM) clamps the whole NC when
            # parallel engine density spikes, so spreading the mask across
            # ACT+DVE measures slower end-to-end than this single op.
            a_t = wrk.tile([P, CHUNK_JT * ni], mm_dt, tag="a")
            nc.vector.scalar_tensor_tensor(
                out=a_t[:, : ln * ni],
                in0=td_t[:, : ln * ni],
                scalar=float(thr) + 0.5,
                in1=adj_t[:, : ln * ni],
                op0=mybir.AluOpType.is_lt,
                op1=mybir.AluOpType.mult,
            )

            for jl in range(ln):
                jt = j0 + jl
                lhsT = x_t[:, jt * P : (jt + 1) * P]
                for h in range(nh):
                    nc.tensor.matmul(
                        psum_main[:, h * mm_n : (h + 1) * mm_n],
                        lhsT,
                        a_t[:, jl * ni + h * mm_n : jl * ni + (h + 1) * mm_n],
                        start=(jt == 0),
                        stop=(jt == jt_n - 1),
                    )

        # Per-node MLP tail: full-width elementwise ops (fewer cross-engine
        # dependency hops), matmuls still per-PSUM-bank halves.
        h_ps = pp.tile([P, ni], f32, tag="hps")
        o_ps = pp.tile([P, ni], f32, tag="ops")
        res_t = singles.tile([P, ni], mm_dt)
        nc.vector.tensor_copy(res_t, psum_main)
        for h in range(nh):
            hs = slice(h * mm_n, (h + 1) * mm_n)
            nc.tensor.matmul(h_ps[:, hs], bd1_t, res_t[:, hs], start=True, stop=True)
        # h = relu(. + b1) fused on DVE: (in + bias) max 0
        h_t = singles.tile([P, ni], mm_dt)
        nc.vector.tensor_scalar(
            h_t, h_ps, bias1_t, 0.0,
            op0=mybir.AluOpType.add,
            op1=mybir.AluOpType.max,
        )
        for h in range(nh):
            hs = slice(h * mm_n, (h + 1) * mm_n)
            nc.tensor.matmul(o_ps[:, hs], bd2_t, h_t[:, hs], start=True, stop=True)
        out_t = singles.tile([P, ni], f32)
        nc.scalar.activation(
            out_t, o_ps, mybir.ActivationFunctionType.Sigmoid, bias=bias2_t
        )
        nc.sync.dma_start(out=outT_d, in_=out_t)

    nc.compile()
    return nc


def _host_prep(x, adj, time_delay, t, W1, b1, W2, b2, ncores, mm_np, rnd, td_dtype):
    """Layout-only transforms (transpose / repack / dtype container changes)."""
    x = np.ascontiguousarray(np.asarray(x, dtype=np.float32))
    adj = np.asarray(adj, dtype=np.float32)
    td = np.asarray(time_delay)
    b, n, f = x.shape
    ni = n // ncores
    jt_n = n // P

    thr = int(t) * 5 + 4  # time_delay // 5 <= t  <=>  time_delay <= 5t+4

    # chunk-linear transposed layouts: arr[p, jt, i_global]
    adjT = np.ascontiguousarray(
        rnd(adj.T).reshape(jt_n, P, n).transpose(1, 0, 2)
    )
    tdT = np.ascontiguousarray(
        td.T.astype(td_dtype).reshape(jt_n, P, n).transpose(1, 0, 2)
    )
    # stationary x: x_sb[p, jt*P + 32*b + f] = x[b, jt*P + p, f]
    xsb = rnd(
        x.reshape(b, jt_n, P, f).transpose(2, 1, 0, 3).reshape(P, jt_n * b * f)
    )
    bd1 = np.zeros((P, P), np.float32)
    bd2 = np.zeros((P, P), np.float32)
    for bb in range(b):
        bd1[bb * f : (bb + 1) * f, bb * f : (bb + 1) * f] = W1
        bd2[bb * f : (bb + 1) * f, bb * f : (bb + 1) * f] = W2
    bd1 = rnd(bd1)
    bd2 = rnd(bd2)
    bias1 = np.ascontiguousarray(np.tile(np.asarray(b1, np.float32), b).reshape(P, 1))
    bias2 = np.ascontiguousarray(np.tile(np.asarray(b2, np.float32), b).reshape(P, 1))

    in_maps = []
    for c in range(ncores):
        sl = slice(c * ni, (c + 1) * ni)
        in_maps.append(
            {
                "adjT": np.ascontiguousarray(adjT[:, :, sl]).reshape(P, jt_n * ni),
                "tdT": np.ascontiguousarray(tdT[:, :, sl]).reshape(P, jt_n * ni),
                "xsb": xsb,
                "bd1": bd1,
                "bd2": bd2,
                "bias1": bias1,
                "bias2": bias2,
            }
        )
    return thr, in_maps


def _run(x, adj, time_delay, t, W1, b1, W2, b2, ncores=NCORES,
         mm_dtype_name="float16", trace=False):
    from concourse.bass_utils import run_bass_kernel_spmd

    b, n, f = np.asarray(x).shape
    ni = n // ncores
    td = np.asarray(time_delay)
    # int8 shipping is only a container change; keep int32 when values
    # (or the threshold compare range) would not fit exactly.
    thr_chk = int(t) * 5 + 4
    if td.min() >= -127 and td.max() <= 127 and -127 <= thr_chk <= 127:
        td_dtype = np.int8
    else:
        td_dtype = np.int32
    if mm_dtype_name == "float32r":
        rnd = _round_fp32r
        mm_np = np.float32
    elif mm_dtype_name == "float16":
        mm_np = np.float16
        rnd = lambda a: np.ascontiguousarray(a, dtype=np.float16)
    else:
        mm_np = np.float32
        rnd = lambda a: np.ascontiguousarray(a, dtype=np.float32)
    thr, in_maps = _host_prep(
        x, adj, time_delay, t, W1, b1, W2, b2, ncores, mm_np, rnd, td_dtype
    )
    nc = _build(n, ni, thr, mm_dtype_name, td_dtype)
    res = run_bass_kernel_spmd(
        nc, in_maps, core_ids=list(range(ncores)), trace=trace
    )
    full = np.concatenate([r["outT"] for r in res.results], axis=1)  # [P, n]
    out = np.ascontiguousarray(full.reshape(b, f, n).transpose(0, 2, 1))
    return out, res


def kernel(x, adj, time_delay, t, W1, b1, W2, b2):
    out, _ = _run(x, adj, time_delay, t, W1, b1, W2, b2)
    return out


# revision 16
# speedup vs baseline: 1.1066x; 1.0238x over previous
"""Trainium2 Bass kernel for nn_DGNN (gnn_message_passing).

Reference computation (B=4, N=8192, F=32):
    delay_steps = time_delay // 5
    active      = (t >= delay_steps) & (adj > 0)
    A           = where(active, adj, 0)              # == adj * (time_delay <= 5*t+4)
    adjusted    = einsum('ij,bjf->bif', A, x)
    h           = relu(adjusted @ W1 + b1)
    out         = sigmoid(h @ W2 + b2)

Sharding / layout (host does layout-only transforms + dtype container
changes, no reference math):
  - destination nodes i are split row-wise across 8 cores (1024 each);
  - adj/time_delay are shipped transposed ([j, i], j on partitions) because
    the PE contracts over the partition dim;
  - adj and x are shipped as fp16 (the matmul runs in fp16 with fp32 PSUM
    accumulation; measured end-to-end rel err ~4e-3 vs the 2e-2 gate, and
    it halves both HBM traffic and PE time vs fp32);
  - time_delay values are 0..99 so they are shipped as int8 when they fit
    (lossless narrowing; falls back to int32 otherwise);
  - adjT/tdT are laid out chunk-linear ([P, jt*ni] with whole j-tiles
    contiguous) so each DMA moves multiple j-tiles in one 1-2 MiB
    transfer (large transfers amortize the per-DMA completion latency);
  - x is repacked so the 4 batches sit side-by-side in the stationary
    operand (partition q = 32*b + f), giving full-width M=128 matmuls;
  - W1/W2 become 128x128 block-diagonal so the per-node MLP handles all 4
    batches in one matmul.

On-device per core: stream adjT/tdT chunks double-buffered on the two
HWDGE rings, one fused DVE op per chunk (TENSOR_MASK:
out = select(td < thr+0.5, adj, 0)) produces the masked adjacency in
fp16, fp16 matmuls accumulate adjusted^T over 64 K-tiles in fp32 PSUM,
then the block-diagonal MLP (fp16 matmuls, fp32 bias/activations) and
sigmoid run on-chip. Output returns transposed per core and is unsharded
on the host. The first two chunks are half-size so compute starts ~5 us
earlier in the DMA stream.
"""

import numpy as np

B = 4
N = 8192
F = 32
P = 128
NCORES = 8
NI = N // NCORES  # dest-nodes per core
JT = N // P       # contraction tiles

MM_N = 512        # moving-operand free dim per matmul (one PSUM bank)
CHUNK_JT = 4      # j-tiles per DMA chunk in steady state
IO_BUFS = 4       # in-flight adj/td chunk buffers
A_BUFS = 3        # in-flight masked-adjacency buffers


def _chunk_schedule(jt_n):
    """(start_tile, n_tiles) per DMA chunk: half chunks at both ends (fast
    pipeline ramp, fast drain), full chunks in the middle."""
    half = CHUNK_JT // 2
    sched = []
    pos = 0
    if jt_n >= 4 * CHUNK_JT:
        sched = [(0, half), (half, half)]
        pos = 2 * half
        tail_small = 2
    else:
        tail_small = 0
    end = jt_n - tail_small * half
    while pos < end:
        ln = min(CHUNK_JT, end - pos)
        sched.append((pos, ln))
        pos += ln
    for _ in range(tail_small):
        sched.append((pos, half))
        pos += half
    return sched


def _round_fp32r(a):
    """Round fp32 to the fp32r grid (11 explicit mantissa bits, RNE)."""
    u = np.ascontiguousarray(a, dtype=np.float32).view(np.uint32)
    low = u & np.uint32(0xFFF)
    lsb = (u >> np.uint32(12)) & np.uint32(1)
    roundup = (low > 0x800) | ((low == 0x800) & (lsb == 1))
    u2 = (u & np.uint32(0xFFFFF000)) + (roundup.astype(np.uint32) << np.uint32(12))
    return u2.view(np.float32)


def _build(nj, ni, thr, mm_dtype_name="float16", td_dtype=np.int8):
    """Trace + compile the per-core Bass program."""
    from contextlib import ExitStack

    import concourse.bacc as bacc
    import concourse.mybir as mybir
    import concourse.tile as tile

    f32 = mybir.dt.float32
    mm_dt = getattr(mybir.dt, mm_dtype_name)
    td_dt = mybir.dt.from_np(np.dtype(td_dtype))

    jt_n = nj // P
    mm_n = min(MM_N, ni)
    nh = ni // mm_n
    sched = _chunk_schedule(jt_n)

    nc = bacc.Bacc("TRN2", target_bir_lowering=False, debug=False)

    adjT_d = nc.dram_tensor("adjT", [P, jt_n * ni], mm_dt, kind="ExternalInput").ap()
    tdT_d = nc.dram_tensor("tdT", [P, jt_n * ni], td_dt, kind="ExternalInput").ap()
    xsb_d = nc.dram_tensor("xsb", [P, jt_n * P], mm_dt, kind="ExternalInput").ap()
    bd1_d = nc.dram_tensor("bd1", [P, P], mm_dt, kind="ExternalInput").ap()
    bd2_d = nc.dram_tensor("bd2", [P, P], mm_dt, kind="ExternalInput").ap()
    bias1_d = nc.dram_tensor("bias1", [P, 1], f32, kind="ExternalInput").ap()
    bias2_d = nc.dram_tensor("bias2", [P, 1], f32, kind="ExternalInput").ap()
    outT_d = nc.dram_tensor("outT", [P, ni], f32, kind="ExternalOutput").ap()

    with tile.TileContext(nc) as tc, ExitStack() as ctx:
        io = ctx.enter_context(tc.tile_pool(name="io", bufs=IO_BUFS))
        wrk = ctx.enter_context(tc.tile_pool(name="wrk", bufs=A_BUFS))
        singles = ctx.enter_context(tc.tile_pool(name="singles", bufs=1))
        pp = ctx.enter_context(tc.tile_pool(name="pp", bufs=1, space="PSUM"))

        x_t = singles.tile([P, jt_n * P], mm_dt)
        psum_main = pp.tile([P, ni], f32)
        bd1_t = singles.tile([P, P], mm_dt)
        bd2_t = singles.tile([P, P], mm_dt)
        bias1_t = singles.tile([P, 1], f32)
        bias2_t = singles.tile([P, 1], f32)
        warm_t = singles.tile([P, 1], f32)

        # ACT table pre-warm for the sigmoid (used both as the exact 0/1
        # step builder in the stream and in the tail), before any data
        # lands (no input dependencies).
        bias_thr_t = singles.tile([P, 1], f32)
        nc.vector.memset(bias_thr_t, 100.0 * (float(thr) + 0.5))
        nc.vector.memset(warm_t, 0.0)
        nc.scalar.activation(
            warm_t, warm_t, mybir.ActivationFunctionType.Sigmoid, bias=bias_thr_t
        )

        for c, (j0, ln) in enumerate(sched):
            # all stream DMAs ride the SP HWDGE ring; the ACT ring only
            # carries small constants so the scalar engine stays free for
            # the mask compares
            cs = slice(j0 * ni, (j0 + ln) * ni)
            adj_t = io.tile([P, CHUNK_JT * ni], mm_dt, tag="adj")
            nc.sync.dma_start(out=adj_t[:, : ln * ni], in_=adjT_d[:, cs])
            td_t = io.tile([P, CHUNK_JT * ni], td_dt, tag="td")
            nc.sync.dma_start(out=td_t[:, : ln * ni], in_=tdT_d[:, cs])
            xs = slice(j0 * P, (j0 + ln) * P)
            nc.sync.dma_start(out=x_t[:, xs], in_=xsb_d[:, xs])
            if c == 1:
                # small constants, off the critical path
                nc.scalar.dma_start(out=bd1_t, in_=bd1_d)
                nc.scalar.dma_start(out=bd2_t, in_=bd2_d)
                nc.scalar.dma_start(out=bias1_t, in_=bias1_d)
                nc.scalar.dma_start(out=bias2_t, in_=bias2_d)

            # A = (td < thr + 0.5) * adj in one fused DVE op. Keeping the
            # mask as a single 1x op minimizes total engine activity: the
            # chip's activity throttle (HAM) clamps the whole NC when
            # parallel engine density spikes, so spreading the mask across
            # ACT+DVE measures slower end-to-end than this single op.
            a_t = wrk.tile([P, CHUNK_JT * ni], mm_dt, tag="a")
            nc.vector.scalar_tensor_tensor(
                out=a_t[:, : ln * ni],
                in0=td_t[:, : ln * ni],
                scalar=float(thr) + 0.5,
                in1=adj_t[:, : ln * ni],
                op0=mybir.AluOpType.is_lt,
                op1=mybir.AluOpType.mult,
            )

            for jl in range(ln):
                jt = j0 + jl
                lhsT = x_t[:, jt * P : (jt + 1) * P]
                for h in range(nh):
                    nc.tensor.matmul(
                        psum_main[:, h * mm_n : (h + 1) * mm_n],
                        lhsT,
                        a_t[:, jl * ni + h * mm_n : jl * ni + (h + 1) * mm_n],
                        start=(jt == 0),
                        stop=(jt == jt_n - 1),
                    )

        # Per-node MLP tail: full-width elementwise ops (fewer cross-engine
        # dependency hops), matmuls still per-PSUM-bank halves.
        h_ps = pp.tile([P, ni], f32, tag="hps")
        o_ps = pp.tile([P, ni], f32, tag="ops")
        res_t = singles.tile([P, ni], mm_dt)
        nc.vector.tensor_copy(res_t, psum_main)
        for h in range(nh):
            hs = slice(h * mm_n, (h + 1) * mm_n)
            nc.tensor.matmul(h_ps[:, hs], bd1_t, res_t[:, hs], start=True, stop=True)
        # h = relu(. + b1) fused on DVE: (in + bias) max 0
        h_t = singles.tile([P, ni], mm_dt)
        nc.vector.tensor_scalar(
            h_t, h_ps, bias1_t, 0.0,
            op0=mybir.AluOpType.add,
            op1=mybir.AluOpType.max,
        )
        for h in range(nh):
            hs = slice(h * mm_n, (h + 1) * mm_n)
            nc.tensor.matmul(o_ps[:, hs], bd2_t, h_t[:, hs], start=True, stop=True)
        out_t = singles.tile([P, ni], f32)
        nc.scalar.activation(
            out_t, o_ps, mybir.ActivationFunctionType.Sigmoid, bias=bias2_t
        )
        nc.sync.dma_start(out=outT_d, in_=out_t)

    nc.compile()
    return nc


def _host_prep(x, adj, time_delay, t, W1, b1, W2, b2, ncores, mm_np, rnd, td_dtype):
    """Layout-only transforms (transpose / repack / dtype container changes)."""
    x = np.ascontiguousarray(np.asarray(x, dtype=np.float32))
    adj = np.asarray(adj, dtype=np.float32)
    td = np.asarray(time_delay)
    b, n, f = x.shape
    ni = n // ncores
    jt_n = n // P

    thr = int(t) * 5 + 4  # time_delay // 5 <= t  <=>  time_delay <= 5t+4

    # chunk-linear transposed layouts: arr[p, jt, i_global]
    adjT = np.ascontiguousarray(
        rnd(adj.T).reshape(jt_n, P, n).transpose(1, 0, 2)
    )
    tdT = np.ascontiguousarray(
        td.T.astype(td_dtype).reshape(jt_n, P, n).transpose(1, 0, 2)
    )
    # stationary x: x_sb[p, jt*P + 32*b + f] = x[b, jt*P + p, f]
    xsb = rnd(
        x.reshape(b, jt_n, P, f).transpose(2, 1, 0, 3).reshape(P, jt_n * b * f)
    )
    bd1 = np.zeros((P, P), np.float32)
    bd2 = np.zeros((P, P), np.float32)
    for bb in range(b):
        bd1[bb * f : (bb + 1) * f, bb * f : (bb + 1) * f] = W1
        bd2[bb * f : (bb + 1) * f, bb * f : (bb + 1) * f] = W2
    bd1 = rnd(bd1)
    bd2 = rnd(bd2)
    bias1 = np.ascontiguousarray(np.tile(np.asarray(b1, np.float32), b).reshape(P, 1))
    bias2 = np.ascontiguousarray(np.tile(np.asarray(b2, np.float32), b).reshape(P, 1))

    in_maps = []
    for c in range(ncores):
        sl = slice(c * ni, (c + 1) * ni)
        in_maps.append(
            {
                "adjT": np.ascontiguousarray(adjT[:, :, sl]).reshape(P, jt_n * ni),
                "tdT": np.ascontiguousarray(tdT[:, :, sl]).reshape(P, jt_n * ni),
                "xsb": xsb,
                "bd1": bd1,
                "bd2": bd2,
                "bias1": bias1,
                "bias2": bias2,
            }
        )
    return thr, in_maps


def _run(x, adj, time_delay, t, W1, b1, W2, b2, ncores=NCORES,
         mm_dtype_name="float16", trace=False):
    from concourse.bass_utils import run_bass_kernel_spmd

    b, n, f = np.asarray(x).shape
    ni = n // ncores
    td = np.asarray(time_delay)
    # int8 shipping is only a container change; keep int32 when values
    # (or the threshold compare range) would not fit exactly.
    thr_chk = int(t) * 5 + 4
    if td.min() >= -127 and td.max() <= 127 and -127 <= thr_chk <= 127:
        td_dtype = np.int8
    else:
        td_dtype = np.int32
    if mm_dtype_name == "float32r":
        rnd = _round_fp32r
        mm_np = np.float32
    elif mm_dtype_name == "float16":
        mm_np = np.float16
        rnd = lambda a: np.ascontiguousarray(a, dtype=np.float16)
    else:
        mm_np = np.float32
        rnd = lambda a: np.ascontiguousarray(a, dtype=np.float32)
    thr, in_maps = _host_prep(
        x, adj, time_delay, t, W1, b1, W2, b2, ncores, mm_np, rnd, td_dtype
    )
    nc = _build(n, ni, thr, mm_dtype_name, td_dtype)
    res = run_bass_kernel_spmd(
        nc, in_maps, core_ids=list(range(ncores)), trace=trace
    )
    full = np.concatenate([r["outT"] for r in res.results], axis=1)  # [P, n]
    out = np.ascontiguousarray(full.reshape(b, f, n).transpose(0, 2, 1))
    return out, res


def kernel(x, adj, time_delay, t, W1, b1, W2, b2):
    out, _ = _run(x, adj, time_delay, t, W1, b1, W2, b2)
    return out


# revision 22
# speedup vs baseline: 1.1421x; 1.0321x over previous
"""Trainium2 Bass kernel for nn_DGNN (gnn_message_passing).

Reference computation (B=4, N=8192, F=32):
    delay_steps = time_delay // 5
    active      = (t >= delay_steps) & (adj > 0)
    A           = where(active, adj, 0)              # == adj * (time_delay <= 5*t+4)
    adjusted    = einsum('ij,bjf->bif', A, x)
    h           = relu(adjusted @ W1 + b1)
    out         = sigmoid(h @ W2 + b2)

Sharding / layout (host does layout-only transforms + dtype container
changes, no reference math):
  - destination nodes i are split row-wise across 8 cores (1024 each);
  - adj/time_delay are shipped transposed ([j, i], j on partitions) because
    the PE contracts over the partition dim;
  - adj and x are shipped as fp16 (the matmul runs in fp16 with fp32 PSUM
    accumulation; measured end-to-end rel err 7.5e-3 vs the 2e-2 gate, and
    it halves both HBM traffic and PE time vs fp32; bf16 fails the gate at
    3.1e-2, fp8 fails at 0.4);
  - time_delay values are 0..99 so they are shipped as int8 when they fit
    (lossless narrowing; falls back to int32 otherwise);
  - adjT/tdT are laid out chunk-linear ([P, jt*ni] with whole j-tiles
    contiguous) so each DMA moves 4 j-tiles in one 0.5-1 MiB transfer
    (large transfers amortize the per-DMA completion latency);
  - x is repacked so the 4 batches sit side-by-side in the stationary
    operand (partition q = 32*b + f), giving full-width M=128 matmuls;
  - W1/W2 become 128x128 block-diagonal so the per-node MLP handles all 4
    batches in one matmul.

On-device per core: adjT/tdT/x chunks stream 4-deep on the SP HWDGE ring
(measured faster than splitting across both rings); per chunk one fused
DVE ScalarTensorTensor op computes A = (td < thr+0.5) * adj in fp16
(~68 us serial on DVE for the whole stream, just under the ~75 us DMA
stream - the two pace each other); fp16 matmuls accumulate adjusted^T
over 64 K-tiles in fp32 PSUM; the block-diagonal MLP (fp16 matmuls,
fp32 bias/relu/sigmoid, full-width elementwise ops) runs on-chip and a
single DMA returns outT. Output is unsharded on the host. The chunk
schedule is half-size at both ends (fast ramp, fast drain).

Perf notes from trace iteration (measured on trn2, 8-core SPMD):
  - run-to-run exec noise is +-5 us; judge changes by min-of-3;
  - DVE builtin ops run 1 elem/cycle/partition (~123 G elem/s); the 2x
    packed mode needs ALL non-scalar operands 2-byte (int8 td blocks it);
    splitting compare onto ACT + 2x TensorTensor multiply on DVE measured
    no better end-to-end (extra cross-engine handoffs + total activity);
  - HAM throttles the PE to 4/8 pulses whenever matmul bursts alternate
    with idle, but PE is never the critical path here;
  - fp32 everything (baseline) was 147 us: simultaneously PE-bound
    (fp32 matmul at quarter rate) and DMA-bound (2x adj bytes).
"""

import numpy as np

B = 4
N = 8192
F = 32
P = 128
NCORES = 8
NI = N // NCORES  # dest-nodes per core
JT = N // P       # contraction tiles

MM_N = 512        # moving-operand free dim per matmul (one PSUM bank)
CHUNK_JT = 4      # j-tiles per DMA chunk in steady state
IO_BUFS = 4       # in-flight adj/td chunk buffers
A_BUFS = 4        # in-flight masked-adjacency buffers


def _chunk_schedule(jt_n):
    """(start_tile, n_tiles) per DMA chunk: half chunks at both ends (fast
    pipeline ramp, fast drain), full chunks in the middle."""
    half = CHUNK_JT // 2
    sched = []
    pos = 0
    if jt_n >= 4 * CHUNK_JT:
        sched = [(0, half), (half, half)]
        pos = 2 * half
        tail_small = 2
    else:
        tail_small = 0
    end = jt_n - tail_small * half
    while pos < end:
        ln = min(CHUNK_JT, end - pos)
        sched.append((pos, ln))
        pos += ln
    for _ in range(tail_small):
        sched.append((pos, half))
        pos += half
    return sched


def _round_fp32r(a):
    """Round fp32 to the fp32r grid (11 explicit mantissa bits, RNE)."""
    u = np.ascontiguousarray(a, dtype=np.float32).view(np.uint32)
    low = u & np.uint32(0xFFF)
    lsb = (u >> np.uint32(12)) & np.uint32(1)
    roundup = (low > 0x800) | ((low == 0x800) & (lsb == 1))
    u2 = (u & np.uint32(0xFFFFF000)) + (roundup.astype(np.uint32) << np.uint32(12))
    return u2.view(np.float32)


def _build(nj, ni, thr, mm_dtype_name="float16", td_dtype=np.int8):
    """Trace + compile the per-core Bass program."""
    from contextlib import ExitStack

    import concourse.bacc as bacc
    import concourse.mybir as mybir
    import concourse.tile as tile

    f32 = mybir.dt.float32
    mm_dt = getattr(mybir.dt, mm_dtype_name)
    td_dt = mybir.dt.from_np(np.dtype(td_dtype))

    jt_n = nj // P
    mm_n = min(MM_N, ni)
    nh = ni // mm_n
    sched = _chunk_schedule(jt_n)

    nc = bacc.Bacc("TRN2", target_bir_lowering=False, debug=False)

    adjT_d = nc.dram_tensor("adjT", [P, jt_n * ni], mm_dt, kind="ExternalInput").ap()
    tdT_d = nc.dram_tensor("tdT", [P, jt_n * ni], td_dt, kind="ExternalInput").ap()
    xsb_d = nc.dram_tensor("xsb", [P, jt_n * P], mm_dt, kind="ExternalInput").ap()
    bd1_d = nc.dram_tensor("bd1", [P, P], mm_dt, kind="ExternalInput").ap()
    bd2_d = nc.dram_tensor("bd2", [P, P], mm_dt, kind="ExternalInput").ap()
    bias1_d = nc.dram_tensor("bias1", [P, 1], f32, kind="ExternalInput").ap()
    bias2_d = nc.dram_tensor("bias2", [P, 1], f32, kind="ExternalInput").ap()
    outT_d = nc.dram_tensor("outT", [P, ni], f32, kind="ExternalOutput").ap()

    with tile.TileContext(nc) as tc, ExitStack() as ctx:
        io = ctx.enter_context(tc.tile_pool(name="io", bufs=IO_BUFS))
        wrk = ctx.enter_context(tc.tile_pool(name="wrk", bufs=A_BUFS))
        singles = ctx.enter_context(tc.tile_pool(name="singles", bufs=1))
        pp = ctx.enter_context(tc.tile_pool(name="pp", bufs=1, space="PSUM"))

        x_t = singles.tile([P, jt_n * P], mm_dt)
        psum_main = pp.tile([P, ni], f32)
        bd1_t = singles.tile([P, P], mm_dt)
        bd2_t = singles.tile([P, P], mm_dt)
        bias1_t = singles.tile([P, 1], f32)
        bias2_t = singles.tile([P, 1], f32)
        warm_t = singles.tile([P, 1], f32)

        # ACT table pre-warm for the sigmoid (used both as the exact 0/1
        # step builder in the stream and in the tail), before any data
        # lands (no input dependencies).
        bias_thr_t = singles.tile([P, 1], f32)
        nc.vector.memset(bias_thr_t, 100.0 * (float(thr) + 0.5))
        nc.vector.memset(warm_t, 0.0)
        nc.scalar.activation(
            warm_t, warm_t, mybir.ActivationFunctionType.Sigmoid, bias=bias_thr_t
        )

        for c, (j0, ln) in enumerate(sched):
            # all stream DMAs ride one HWDGE ring (measured faster than
            # splitting across both rings for this stream)
            cs = slice(j0 * ni, (j0 + ln) * ni)
            adj_t = io.tile([P, CHUNK_JT * ni], mm_dt, tag="adj")
            nc.sync.dma_start(out=adj_t[:, : ln * ni], in_=adjT_d[:, cs])
            td_t = io.tile([P, CHUNK_JT * ni], td_dt, tag="td")
            nc.sync.dma_start(out=td_t[:, : ln * ni], in_=tdT_d[:, cs])
            xs = slice(j0 * P, (j0 + ln) * P)
            nc.sync.dma_start(out=x_t[:, xs], in_=xsb_d[:, xs])
            if c == 1:
                # small constants, off the critical path
                nc.scalar.dma_start(out=bd1_t, in_=bd1_d)
                nc.scalar.dma_start(out=bd2_t, in_=bd2_d)
                nc.scalar.dma_start(out=bias1_t, in_=bias1_d)
                nc.scalar.dma_start(out=bias2_t, in_=bias2_d)

            # A = (td < thr + 0.5) * adj in one fused DVE op. Keeping the
            # mask as a single 1x op minimizes total engine activity: the
            # chip's activity throttle (HAM) clamps the whole NC when
            # parallel engine density spikes, so spreading the mask across
            # ACT+DVE measures slower end-to-end than this single op.
            a_t = wrk.tile([P, CHUNK_JT * ni], mm_dt, tag="a")
            nc.vector.scalar_tensor_tensor(
                out=a_t[:, : ln * ni],
                in0=td_t[:, : ln * ni],
                scalar=float(thr) + 0.5,
                in1=adj_t[:, : ln * ni],
                op0=mybir.AluOpType.is_lt,
                op1=mybir.AluOpType.mult,
            )

            for jl in range(ln):
                jt = j0 + jl
                lhsT = x_t[:, jt * P : (jt + 1) * P]
                for h in range(nh):
                    nc.tensor.matmul(
                        psum_main[:, h * mm_n : (h + 1) * mm_n],
                        lhsT,
                        a_t[:, jl * ni + h * mm_n : jl * ni + (h + 1) * mm_n],
                        start=(jt == 0),
                        stop=(jt == jt_n - 1),
                    )

        # Per-node MLP tail: full-width elementwise ops (fewer cross-engine
        # dependency hops), matmuls still per-PSUM-bank halves.
        h_ps = pp.tile([P, ni], f32, tag="hps")
        o_ps = pp.tile([P, ni], f32, tag="ops")
        res_t = singles.tile([P, ni], mm_dt)
        nc.vector.tensor_copy(res_t, psum_main)
        for h in range(nh):
            hs = slice(h * mm_n, (h + 1) * mm_n)
            nc.tensor.matmul(h_ps[:, hs], bd1_t, res_t[:, hs], start=True, stop=True)
        # h = relu(. + b1) fused on DVE: (in + bias) max 0
        h_t = singles.tile([P, ni], mm_dt)
        nc.vector.tensor_scalar(
            h_t, h_ps, bias1_t, 0.0,
            op0=mybir.AluOpType.add,
            op1=mybir.AluOpType.max,
        )
        for h in range(nh):
            hs = slice(h * mm_n, (h + 1) * mm_n)
            nc.tensor.matmul(o_ps[:, hs], bd2_t, h_t[:, hs], start=True, stop=True)
        out_t = singles.tile([P, ni], f32)
        nc.scalar.activation(
            out_t, o_ps, mybir.ActivationFunctionType.Sigmoid, bias=bias2_t
        )
        nc.sync.dma_start(out=outT_d, in_=out_t)

    nc.compile()
    return nc


def _host_prep(x, adj, time_delay, t, W1, b1, W2, b2, ncores, mm_np, rnd, td_dtype):
    """Layout-only transforms (transpose / repack / dtype container changes)."""
    x = np.ascontiguousarray(np.asarray(x, dtype=np.float32))
    adj = np.asarray(adj, dtype=np.float32)
    td = np.asarray(time_delay)
    b, n, f = x.shape
    ni = n // ncores
    jt_n = n // P

    thr = int(t) * 5 + 4  # time_delay // 5 <= t  <=>  time_delay <= 5t+4

    # chunk-linear transposed layouts: arr[p, jt, i_global]
    adjT = np.ascontiguousarray(
        rnd(adj.T).reshape(jt_n, P, n).transpose(1, 0, 2)
    )
    tdT = np.ascontiguousarray(
        td.T.astype(td_dtype).reshape(jt_n, P, n).transpose(1, 0, 2)
    )
    # stationary x: x_sb[p, jt*P + 32*b + f] = x[b, jt*P + p, f]
    xsb = rnd(
        x.reshape(b, jt_n, P, f).transpose(2, 1, 0, 3).reshape(P, jt_n * b * f)
    )
    bd1 = np.zeros((P, P), np.float32)
    bd2 = np.zeros((P, P), np.float32)
    for bb in range(b):
        bd1[bb * f : (bb + 1) * f, bb * f : (bb + 1) * f] = W1
        bd2[bb * f : (bb + 1) * f, bb * f : (bb + 1) * f] = W2
    bd1 = rnd(bd1)
    bd2 = rnd(bd2)
    bias1 = np.ascontiguousarray(np.tile(np.asarray(b1, np.float32), b).reshape(P, 1))
    bias2 = np.ascontiguousarray(np.tile(np.asarray(b2, np.float32), b).reshape(P, 1))

    in_maps = []
    for c in range(ncores):
        sl = slice(c * ni, (c + 1) * ni)
        in_maps.append(
            {
                "adjT": np.ascontiguousarray(adjT[:, :, sl]).reshape(P, jt_n * ni),
                "tdT": np.ascontiguousarray(tdT[:, :, sl]).reshape(P, jt_n * ni),
                "xsb": xsb,
                "bd1": bd1,
                "bd2": bd2,
                "bias1": bias1,
                "bias2": bias2,
            }
        )
    return thr, in_maps


def _run(x, adj, time_delay, t, W1, b1, W2, b2, ncores=NCORES,
         mm_dtype_name="float16", trace=False):
    from concourse.bass_utils import run_bass_kernel_spmd

    b, n, f = np.asarray(x).shape
    ni = n // ncores
    td = np.asarray(time_delay)
    # int8 shipping is only a container change; keep int32 when values
    # (or the threshold compare range) would not fit exactly.
    thr_chk = int(t) * 5 + 4
    if td.min() >= -127 and td.max() <= 127 and -127 <= thr_chk <= 127:
        td_dtype = np.int8
    else:
        td_dtype = np.int32
    if mm_dtype_name == "float32r":
        rnd = _round_fp32r
        mm_np = np.float32
    elif mm_dtype_name == "float16":
        mm_np = np.float16
        rnd = lambda a: np.ascontiguousarray(a, dtype=np.float16)
    else:
        mm_np = np.float32
        rnd = lambda a: np.ascontiguousarray(a, dtype=np.float32)
    thr, in_maps = _host_prep(
        x, adj, time_delay, t, W1, b1, W2, b2, ncores, mm_np, rnd, td_dtype
    )
    nc = _build(n, ni, thr, mm_dtype_name, td_dtype)
    res = run_bass_kernel_spmd(
        nc, in_maps, core_ids=list(range(ncores)), trace=trace
    )
    full = np.concatenate([r["outT"] for r in res.results], axis=1)  # [P, n]
    out = np.ascontiguousarray(full.reshape(b, f, n).transpose(0, 2, 1))
    return out, res


def kernel(x, adj, time_delay, t, W1, b1, W2, b2):
    out, _ = _run(x, adj, time_delay, t, W1, b1, W2, b2)
    return out
